# revision 7
# baseline (speedup 1.0000x reference)
"""Trainium2 Bass kernel for batched attention.

Problem: b=16 batches of softmax(Q K^T / sqrt(128)) V with n=m=2048, d=dv=128,
fp32 inputs/outputs. Sharded: batch dim across 8 NeuronCores (2 per core).

Per-core structure (2 batches, software-pipelined):
  - Q, K loaded fp32->fp16 (SWDGE cast), transposed to [d, seq] layout by the
    DMA xbar transpose engine (no PE or DVE involvement).
  - MM1: S^T chunk [m-tile, 512 n] = (K^T tile)-stationary x Q^T-moving, fp16,
    fp32 PSUM, 1024-col chunks per (c, n-half).
  - exp: mostly ACT (table exp, temperature scale fused); for batch 0 a subset
    of chunks runs on DVE as a one-instruction Schraudolph approximation
    (int16(S*a+b) bitcast to fp16) to keep ACT off the critical path during
    phase A. P^T stored fp16.
  - MM2 per n-tile t: PSUM chains over m-tiles with the [V | ones] fp16 moving
    operand; col 128 = softmax denominator. First-half chains (c=0..7) run
    inside the producing batch's own exp window -> fp16 partials in SBUF;
    second half + merge + normalize run in the next window (batch 1: drain).
  - PE warm-up dummies during the initial DMA raise HAM to 8/8 early.
"""

import math
import numpy as np

B = 16
N_CORES = 8
B_LOC = B // N_CORES  # 2 batches per core
N = 2048  # queries per batch
M = 2048  # keys per batch
D = 128   # head dim
NT = N // 128  # 16 n-tiles
MT = M // 128  # 16 m-tiles
TEMP = 11.313708498984761
INV_TEMP = 1.0 / TEMP

# Schraudolph exp on DVE: bits16 = int16(S * SCH_A + SCH_B); bitcast fp16.
SCH_A = 1024.0 / math.log(2.0) / TEMP
SCH_C = -45.0
SCH_B = 15360.0 + SCH_C

# (c, h) half-chunks of batch 0 handled by DVE-Schraudolph instead of ACT exp.
DVE_CH = {(c, 1) for c in range(1, 16, 2)}

_CACHE = {}


def _build():
    import concourse.bacc as bacc
    import concourse.mybir as mybir
    import concourse.tile as tile

    f32 = mybir.dt.float32
    f16 = mybir.dt.float16
    i16 = mybir.dt.int16

    nc = bacc.Bacc("TRN2", target_bir_lowering=False, debug=False,
                   num_devices=N_CORES)
    q_dram = nc.dram_tensor("queries", [B_LOC, N, D], f32, kind="ExternalInput")
    k_dram = nc.dram_tensor("keys", [B_LOC, M, D], f32, kind="ExternalInput")
    v_dram = nc.dram_tensor("values", [B_LOC, M, D], f32, kind="ExternalInput")
    o_dram = nc.dram_tensor("out", [B_LOC, N, D], f32, kind="ExternalOutput")

    with tile.TileContext(nc) as tc:
        with (
            tc.tile_pool(name="warm", bufs=1) as warm_pool,
            tc.tile_pool(name="nat", bufs=2) as nat_pool,
            tc.tile_pool(name="qT", bufs=2) as qT_pool,
            tc.tile_pool(name="kT", bufs=2) as kT_pool,
            tc.tile_pool(name="vo", bufs=2) as vo_pool,
            tc.tile_pool(name="pT", bufs=32) as pT_pool,
            tc.tile_pool(name="oall", bufs=2) as o_pool,
            tc.tile_pool(name="small", bufs=8) as small_pool,
            tc.tile_pool(name="pa", bufs=32) as pa_pool,
            tc.tile_pool(name="psS", bufs=2, space="PSUM") as psS_pool,
            tc.tile_pool(name="psO", bufs=4, space="PSUM") as psO_pool,
        ):
            # ---- PE warm-up + ACT table primer during the DMA lead-in
            warm = warm_pool.tile([128, 128], f16)
            nc.vector.memset(warm[:], 0.0)
            primer = small_pool.tile([128, 1], f16, tag="prim")
            nc.scalar.activation(primer[:], warm[:, 0:1],
                                 mybir.ActivationFunctionType.Exp,
                                 scale=INV_TEMP)
            psW = psO_pool.tile([128, 128], f32, tag="psO")
            for _ in range(36):
                nc.tensor.matmul(psW[:], warm[:], warm[:], start=True, stop=True)

            def load_nat(dst, srcd, b, order):
                # one 512-col quarter-group (4 seq-tiles), fp32 -> fp16 cast
                cs = slice(order * 4, (order + 1) * 4)
                nc.gpsimd.dma_start(
                    dst[:].rearrange("p (c d) -> p c d", d=128)[:, cs],
                    srcd[b].rearrange("(c p) d -> p c d", p=128)[:, cs])

            def transpose_half(dstT, src_nat, half):
                # xbar transpose of 8 seq-tiles: dst[d, c, n'] = src[n', c*128+d]
                nc.sync.dma_start_transpose(
                    dstT[:].rearrange("p (c n) -> p c n", n=128)
                    [:, half * 8:(half + 1) * 8],
                    src_nat[:, half * 1024:(half + 1) * 1024])

            def load_qk(b):
                q_nat = nat_pool.tile([128, N], f16, tag="nat")
                k_nat = nat_pool.tile([128, M], f16, tag="nat")
                qT = qT_pool.tile([128, N], f16)
                kT = kT_pool.tile([128, M], f16)
                for g in (0, 1):
                    load_nat(k_nat, k_dram, b, g)
                    load_nat(q_nat, q_dram, b, g)
                transpose_half(kT, k_nat, 0)
                transpose_half(qT, q_nat, 0)
                for g in (2, 3):
                    load_nat(k_nat, k_dram, b, g)
                    load_nat(q_nat, q_dram, b, g)
                transpose_half(kT, k_nat, 1)
                transpose_half(qT, q_nat, 1)
                return qT, kT

            def load_v(b):
                vo = vo_pool.tile([128, MT * 129], f16)
                nc.gpsimd.dma_start(
                    vo[:].rearrange("p (c w) -> p c w", w=129)[:, :, 0:128],
                    v_dram[b].rearrange("(c p) d -> p c d", p=128))
                nc.vector.memset(
                    vo[:].rearrange("p (c w) -> p c w", w=129)[:, :, 128:129], 1.0)
                return vo

            def mm1_exp(qT, kT, pT, c, h, dve):
                psS = psS_pool.tile([128, 1024], f32, tag="psS")
                for j in range(2):
                    nc.tensor.matmul(
                        psS[:, j * 512:(j + 1) * 512],
                        kT[:, c * 128:(c + 1) * 128],
                        qT[:, h * 1024 + j * 512:h * 1024 + (j + 1) * 512],
                        start=True, stop=True)
                dst = pT[:, h * 1024:(h + 1) * 1024]
                if dve:
                    nc.vector.tensor_scalar(
                        dst.bitcast(i16), psS[:], SCH_A, SCH_B,
                        mybir.AluOpType.mult, mybir.AluOpType.add)
                else:
                    nc.scalar.activation(
                        dst, psS[:], mybir.ActivationFunctionType.Exp,
                        scale=INV_TEMP)

            def mm2_a(pTs, vo, t):
                # first half: c=0..7 -> fp16 partial in SBUF
                psO = psO_pool.tile([128, 129], f32, tag="psO")
                for c in range(8):
                    nc.tensor.matmul(
                        psO[:], pTs[c][:, t * 128:(t + 1) * 128],
                        vo[:, c * 129:(c + 1) * 129],
                        start=(c == 0), stop=(c == 7))
                pa = pa_pool.tile([128, 129], f16, tag="pa")
                nc.vector.tensor_copy(pa[:], psO[:])
                return pa

            def mm2_b(pTs, vo, o_all, pa, t):
                # second half c=8..15 + merge + normalize
                psO = psO_pool.tile([128, 129], f32, tag="psO")
                for c in range(8, MT):
                    nc.tensor.matmul(
                        psO[:], pTs[c][:, t * 128:(t + 1) * 128],
                        vo[:, c * 129:(c + 1) * 129],
                        start=(c == 8), stop=(c == MT - 1))
                osum = small_pool.tile([128, 129], f32, tag="osum")
                nc.vector.tensor_add(osum[:], psO[:], pa[:])
                recip = small_pool.tile([128, 1], f32, tag="recip")
                nc.vector.reciprocal(recip[:], osum[:, 128:129])
                nc.vector.tensor_scalar_mul(
                    o_all[:, t * 128:(t + 1) * 128], osum[:, 0:128], recip[:])

            def store_out(b, o_all, group):
                cs = slice(group * 4, (group + 1) * 4)
                nc.sync.dma_start(
                    o_dram[b].rearrange("(c p) d -> p c d", p=128)[:, cs],
                    o_all[:].rearrange("p (c d) -> p c d", d=128)[:, cs])

            # ============ phase A: b0 MM1 + exp + first-half MM2 ==========
            qT0, kT0 = load_qk(0)
            pTs0, pas0 = [], {}
            vo0 = None
            for c in range(MT):
                pT = pT_pool.tile([128, N], f16, tag="pT")
                pTs0.append(pT)
                mm1_exp(qT0, kT0, pT, c, 0, dve=(c, 0) in DVE_CH)
                mm1_exp(qT0, kT0, pT, c, 1, dve=(c, 1) in DVE_CH)
                if c == 5:
                    vo0 = load_v(0)
                if c >= 8:
                    for k2 in range(2):
                        t = (c - 8) * 2 + k2
                        pas0[t] = mm2_a(pTs0, vo0, t)
                if c == 11:
                    qT1, kT1 = load_qk(1)

            # ===== phase B: b1 MM1/exp + b0 second-half MM2 + b1 mm2_a ====
            o_all0 = o_pool.tile([128, NT * 128], f32)
            pTs1, pas1 = [], {}
            vo1 = None
            for c in range(MT):
                pT = pT_pool.tile([128, N], f16, tag="pT")
                pTs1.append(pT)
                mm1_exp(qT1, kT1, pT, c, 0, dve=False)
                mm1_exp(qT1, kT1, pT, c, 1, dve=False)
                mm2_b(pTs0, vo0, o_all0, pas0[c], t=c)
                if c % 4 == 3:
                    store_out(0, o_all0, c // 4)
                if c == 5:
                    vo1 = load_v(1)
                if c >= 8:
                    for k2 in range(2):
                        t = (c - 8) * 2 + k2
                        pas1[t] = mm2_a(pTs1, vo1, t)

            # ============ phase C: b1 second-half MM2 drain ===============
            o_all1 = o_pool.tile([128, NT * 128], f32)
            for t in range(NT):
                mm2_b(pTs1, vo1, o_all1, pas1[t], t)
                if t % 4 == 3:
                    store_out(1, o_all1, t // 4)

    nc.compile()
    return nc


def _get_nc():
    if "nc" not in _CACHE:
        _CACHE["nc"] = _build()
    return _CACHE["nc"]


def _ensure_ntff_hook():
    """concourse's trace path imports antenv.axon_hooks, which this image's
    antenv lacks; register an equivalent shim so tracing works."""
    import sys
    try:
        import antenv.axon_hooks  # noqa: F401
        return
    except ImportError:
        pass
    import types
    mod = types.ModuleType("antenv.axon_hooks")
    hook = [None]
    mod.set_axon_ntff_profile_hook = lambda h: hook.__setitem__(0, h)
    mod.get_axon_ntff_profile_hook = lambda: hook[0]
    sys.modules["antenv.axon_hooks"] = mod
    try:
        from trn_agent_boot.trn_boot import _ntff_profile_via_ctypes
        mod.set_axon_ntff_profile_hook(
            _ntff_profile_via_ctypes("/opt/axon/libaxon_pjrt.so"))
    except Exception:
        pass


def run(queries, keys, values, trace=False, tmpdir=None):
    """Run on 8 cores; returns (output, BassKernelResults)."""
    _ensure_ntff_hook()
    from concourse.bass_utils import run_bass_kernel_spmd

    nc = _get_nc()
    queries = np.ascontiguousarray(queries, dtype=np.float32)
    keys = np.ascontiguousarray(keys, dtype=np.float32)
    values = np.ascontiguousarray(values, dtype=np.float32)
    in_maps = []
    for c in range(N_CORES):
        s = slice(c * B_LOC, (c + 1) * B_LOC)
        in_maps.append({
            "queries": queries[s],
            "keys": keys[s],
            "values": values[s],
        })
    res = run_bass_kernel_spmd(nc, in_maps, core_ids=list(range(N_CORES)),
                               trace=trace, tmpdir=tmpdir)
    out = np.concatenate([res.results[c]["out"] for c in range(N_CORES)], axis=0)
    return out, res


def kernel(queries, keys, values):
    out, _ = run(queries, keys, values)
    return out


# revision 12
# speedup vs baseline: 1.1757x; 1.1757x over previous
"""Trainium2 Bass kernel for batched attention.

Problem: b=16 batches of softmax(Q K^T / sqrt(128)) V with n=m=2048, d=dv=128,
fp32 inputs/outputs. Sharded: batch dim across 8 NeuronCores (2 per core).

Per-core structure (2 batches, software-pipelined):
  - Q, K loaded fp32->fp16 (SWDGE cast), transposed to [d, seq] layout by the
    DMA xbar transpose engine (no PE or DVE involvement).
  - MM1: S^T chunk [m-tile, 512 n] = (K^T tile)-stationary x Q^T-moving, fp16,
    fp32 PSUM, 1024-col chunks per (c, n-half).
  - exp: mostly ACT (table exp, temperature scale fused); for batch 0 a subset
    of chunks runs on DVE as a one-instruction Schraudolph approximation
    (int16(S*a+b) bitcast to fp16) to keep ACT off the critical path during
    phase A. P^T stored fp16.
  - MM2 per n-tile t: PSUM chains over m-tiles with the [V | ones] fp16 moving
    operand; col 128 = softmax denominator. First-half chains (c=0..7) run
    inside the producing batch's own exp window -> fp16 partials in SBUF;
    second half + merge + normalize run in the next window (batch 1: drain).
  - PE warm-up dummies during the initial DMA raise HAM to 8/8 early.
"""

import math
import numpy as np

B = 16
N_CORES = 8
B_LOC = B // N_CORES  # 2 batches per core
N = 2048  # queries per batch
M = 2048  # keys per batch
D = 128   # head dim
NT = N // 128  # 16 n-tiles
MT = M // 128  # 16 m-tiles
TEMP = 11.313708498984761
INV_TEMP = 1.0 / TEMP

# Schraudolph exp on DVE: bits16 = int16(S * SCH_A + SCH_B); bitcast fp16.
SCH_A = 1024.0 / math.log(2.0) / TEMP
SCH_C = -45.0
SCH_B = 15360.0 + SCH_C

# (c, h) half-chunks of batch 0 handled by DVE-Schraudolph instead of ACT exp.
DVE_CH = {(c, 1) for c in range(1, 16, 2)}

_CACHE = {}


def _build():
    import concourse.bacc as bacc
    import concourse.mybir as mybir
    import concourse.tile as tile

    f32 = mybir.dt.float32
    f16 = mybir.dt.float16
    i16 = mybir.dt.int16

    nc = bacc.Bacc("TRN2", target_bir_lowering=False, debug=False,
                   num_devices=N_CORES)
    q_dram = nc.dram_tensor("queries", [B_LOC, N, D], f32, kind="ExternalInput")
    k_dram = nc.dram_tensor("keys", [B_LOC, M, D], f32, kind="ExternalInput")
    v_dram = nc.dram_tensor("values", [B_LOC, M, D], f32, kind="ExternalInput")
    o_dram = nc.dram_tensor("out", [B_LOC, N, D], f32, kind="ExternalOutput")

    with tile.TileContext(nc) as tc:
        with (
            tc.tile_pool(name="warm", bufs=1) as warm_pool,
            tc.tile_pool(name="nat", bufs=2) as nat_pool,
            tc.tile_pool(name="qT", bufs=2) as qT_pool,
            tc.tile_pool(name="kT", bufs=2) as kT_pool,
            tc.tile_pool(name="vo", bufs=2) as vo_pool,
            tc.tile_pool(name="pT", bufs=32) as pT_pool,
            tc.tile_pool(name="oall", bufs=2) as o_pool,
            tc.tile_pool(name="small", bufs=8) as small_pool,
            tc.tile_pool(name="pa", bufs=32) as pa_pool,
            tc.tile_pool(name="psS", bufs=2, space="PSUM") as psS_pool,
            tc.tile_pool(name="psO", bufs=4, space="PSUM") as psO_pool,
        ):
            # ---- PE warm-up + ACT table primer during the DMA lead-in
            warm = warm_pool.tile([128, 128], f16)
            nc.vector.memset(warm[:], 0.0)
            primer = small_pool.tile([128, 1], f16, tag="prim")
            nc.scalar.activation(primer[:], warm[:, 0:1],
                                 mybir.ActivationFunctionType.Exp,
                                 scale=INV_TEMP)
            psW = psO_pool.tile([128, 128], f32, tag="psO")
            for _ in range(36):
                nc.tensor.matmul(psW[:], warm[:], warm[:], start=True, stop=True)

            def load_nat(dst, srcd, b, half):
                # one 1024-col half (8 seq-tiles), fp32 -> fp16 cast
                cs = slice(half * 8, (half + 1) * 8)
                nc.gpsimd.dma_start(
                    dst[:].rearrange("p (c d) -> p c d", d=128)[:, cs],
                    srcd[b].rearrange("(c p) d -> p c d", p=128)[:, cs])

            def transpose_half(dstT, src_nat, half):
                # xbar transpose of 8 seq-tiles: dst[d, c, n'] = src[n', c*128+d]
                nc.sync.dma_start_transpose(
                    dstT[:].rearrange("p (c n) -> p c n", n=128)
                    [:, half * 8:(half + 1) * 8],
                    src_nat[:, half * 1024:(half + 1) * 1024])

            def load_qk(b):
                q_nat = nat_pool.tile([128, N], f16, tag="nat")
                k_nat = nat_pool.tile([128, M], f16, tag="nat")
                qT = qT_pool.tile([128, N], f16)
                kT = kT_pool.tile([128, M], f16)
                load_nat(k_nat, k_dram, b, 0)
                load_nat(q_nat, q_dram, b, 0)
                transpose_half(kT, k_nat, 0)
                transpose_half(qT, q_nat, 0)
                load_nat(k_nat, k_dram, b, 1)
                load_nat(q_nat, q_dram, b, 1)
                transpose_half(kT, k_nat, 1)
                transpose_half(qT, q_nat, 1)
                return qT, kT

            def load_v(b):
                vo = vo_pool.tile([128, MT * 129], f16)
                nc.gpsimd.dma_start(
                    vo[:].rearrange("p (c w) -> p c w", w=129)[:, :, 0:128],
                    v_dram[b].rearrange("(c p) d -> p c d", p=128))
                nc.vector.memset(
                    vo[:].rearrange("p (c w) -> p c w", w=129)[:, :, 128:129], 1.0)
                return vo

            def mm1_exp(qT, kT, pT, c, h, dve):
                psS = psS_pool.tile([128, 1024], f32, tag="psS")
                for j in range(2):
                    nc.tensor.matmul(
                        psS[:, j * 512:(j + 1) * 512],
                        kT[:, c * 128:(c + 1) * 128],
                        qT[:, h * 1024 + j * 512:h * 1024 + (j + 1) * 512],
                        start=True, stop=True)
                dst = pT[:, h * 1024:(h + 1) * 1024]
                if dve:
                    nc.vector.tensor_scalar(
                        dst.bitcast(i16), psS[:], SCH_A, SCH_B,
                        mybir.AluOpType.mult, mybir.AluOpType.add)
                else:
                    nc.scalar.activation(
                        dst, psS[:], mybir.ActivationFunctionType.Exp,
                        scale=INV_TEMP)

            def mm2_a(pTs, vo, t):
                # first half: c=0..7 -> fp16 partial in SBUF
                psO = psO_pool.tile([128, 129], f32, tag="psO")
                for c in range(8):
                    nc.tensor.matmul(
                        psO[:], pTs[c][:, t * 128:(t + 1) * 128],
                        vo[:, c * 129:(c + 1) * 129],
                        start=(c == 0), stop=(c == 7))
                pa = pa_pool.tile([128, 129], f16, tag="pa")
                nc.vector.tensor_copy(pa[:], psO[:])
                return pa

            def mm2_b(pTs, vo, o_all, pa, t):
                # second half c=8..15 + merge + normalize
                psO = psO_pool.tile([128, 129], f32, tag="psO")
                for c in range(8, MT):
                    nc.tensor.matmul(
                        psO[:], pTs[c][:, t * 128:(t + 1) * 128],
                        vo[:, c * 129:(c + 1) * 129],
                        start=(c == 8), stop=(c == MT - 1))
                osum = small_pool.tile([128, 129], f32, tag="osum")
                nc.vector.tensor_add(osum[:], psO[:], pa[:])
                recip = small_pool.tile([128, 1], f32, tag="recip")
                nc.vector.reciprocal(recip[:], osum[:, 128:129])
                nc.vector.tensor_scalar_mul(
                    o_all[:, t * 128:(t + 1) * 128], osum[:, 0:128], recip[:])

            def store_out(b, o_all, group):
                cs = slice(group * 4, (group + 1) * 4)
                nc.sync.dma_start(
                    o_dram[b].rearrange("(c p) d -> p c d", p=128)[:, cs],
                    o_all[:].rearrange("p (c d) -> p c d", d=128)[:, cs])

            # ============ phase A: b0 MM1 + exp + first-half MM2 ==========
            # chunk order: h=0 sweeps ahead so the c,h=1 chunks only start
            # once the second transpose halves have landed.
            qT0, kT0 = load_qk(0)
            pTs0 = [pT_pool.tile([128, N], f16, tag="pT", name=f"pT0_{c}")
                    for c in range(MT)]
            pas0 = {}
            order = [(c, 0) for c in range(6)]
            for c in range(6, MT):
                order += [(c, 0), (c - 6, 1)]
            order += [(c, 1) for c in range(10, MT)]
            vo0 = None
            for i, (c, h) in enumerate(order):
                mm1_exp(qT0, kT0, pTs0[c], c, h, dve=(c, h) in DVE_CH)
                if i == 8:
                    vo0 = load_v(0)
                if i == 21:
                    qT1, kT1 = load_qk(1)
                if i >= 22 and i % 2 == 0:
                    # pT tiles 0..7 complete after chunk (7,1) at i=21
                    j = (i - 22) // 2
                    pas0[2 * j] = mm2_a(pTs0, vo0, 2 * j)
                    pas0[2 * j + 1] = mm2_a(pTs0, vo0, 2 * j + 1)
            for t in range(10, NT):
                pas0[t] = mm2_a(pTs0, vo0, t)

            # ===== phase B: b1 MM1/exp + b0 second-half MM2 + b1 mm2_a ====
            o_all0 = o_pool.tile([128, NT * 128], f32)
            pTs1, pas1 = [], {}
            vo1 = None
            for c in range(MT):
                pT = pT_pool.tile([128, N], f16, tag="pT")
                pTs1.append(pT)
                mm1_exp(qT1, kT1, pT, c, 0, dve=False)
                mm1_exp(qT1, kT1, pT, c, 1, dve=False)
                mm2_b(pTs0, vo0, o_all0, pas0[c], t=c)
                if c % 4 == 3:
                    store_out(0, o_all0, c // 4)
                if c == 5:
                    vo1 = load_v(1)
                if c >= 8:
                    for k2 in range(2):
                        t = (c - 8) * 2 + k2
                        pas1[t] = mm2_a(pTs1, vo1, t)

            # ============ phase C: b1 second-half MM2 drain ===============
            o_all1 = o_pool.tile([128, NT * 128], f32)
            for t in range(NT):
                mm2_b(pTs1, vo1, o_all1, pas1[t], t)
                if t % 4 == 3:
                    store_out(1, o_all1, t // 4)

    nc.compile()
    return nc


def _get_nc():
    if "nc" not in _CACHE:
        _CACHE["nc"] = _build()
    return _CACHE["nc"]


def _ensure_ntff_hook():
    """concourse's trace path imports antenv.axon_hooks, which this image's
    antenv lacks; register an equivalent shim so tracing works."""
    import sys
    try:
        import antenv.axon_hooks  # noqa: F401
        return
    except ImportError:
        pass
    import types
    mod = types.ModuleType("antenv.axon_hooks")
    hook = [None]
    mod.set_axon_ntff_profile_hook = lambda h: hook.__setitem__(0, h)
    mod.get_axon_ntff_profile_hook = lambda: hook[0]
    sys.modules["antenv.axon_hooks"] = mod
    try:
        from trn_agent_boot.trn_boot import _ntff_profile_via_ctypes
        mod.set_axon_ntff_profile_hook(
            _ntff_profile_via_ctypes("/opt/axon/libaxon_pjrt.so"))
    except Exception:
        pass


def run(queries, keys, values, trace=False, tmpdir=None):
    """Run on 8 cores; returns (output, BassKernelResults)."""
    _ensure_ntff_hook()
    from concourse.bass_utils import run_bass_kernel_spmd

    nc = _get_nc()
    queries = np.ascontiguousarray(queries, dtype=np.float32)
    keys = np.ascontiguousarray(keys, dtype=np.float32)
    values = np.ascontiguousarray(values, dtype=np.float32)
    in_maps = []
    for c in range(N_CORES):
        s = slice(c * B_LOC, (c + 1) * B_LOC)
        in_maps.append({
            "queries": queries[s],
            "keys": keys[s],
            "values": values[s],
        })
    res = run_bass_kernel_spmd(nc, in_maps, core_ids=list(range(N_CORES)),
                               trace=trace, tmpdir=tmpdir)
    out = np.concatenate([res.results[c]["out"] for c in range(N_CORES)], axis=0)
    return out, res


def kernel(queries, keys, values):
    out, _ = run(queries, keys, values)
    return out
